# revision 4
# baseline (speedup 1.0000x reference)
"""GCN encoder (2x GCNConv + GraphNorm + ReLU + mean-pool) on 8 trn2 cores.

Strategy: graph-aligned node sharding across 8 cores (batch is sorted, so
each core owns a contiguous run of whole graphs -> GraphNorm and pooling are
fully shard-local). Edges are assigned to the core that owns their dst node.
Each core:
  - computes hp = dinv * (x @ W1) for its shard (PE),
  - AllGathers hp so every core holds the full node table,
  - aggregates messages with indirect-DMA gathers (128 rows/call) and
    duplicate-free indirect-DMA scatter-adds into its dst shard,
  - applies symmetric-norm scaling, bias, GraphNorm, ReLU (DVE/ACT/PE),
  - repeats for layer 2, then computes per-graph mean pooling (PE).
Host side: numpy preprocessing (degrees, sharding, edge blocking) + final
assembly of the [128, 64] output.
"""
import numpy as np

import concourse.bass as bass
import concourse.bacc as bacc
import concourse.mybir as mybir
import concourse.tile as tile
from concourse.bass_utils import run_bass_kernel_spmd

dt = mybir.dt

NCORES = 8
NUM_NODES = 100000
NUM_EDGES = 1600000
NUM_GRAPHS = 128
IN_DIM, HID_DIM, EMB_DIM = 128, 64, 64
EPS = 1e-5
GMAX = 32          # max graphs per core (128/8 = 16 avg)
P = 128

_CACHE = {}


def _shard_plan(batch):
    """Graph-aligned contiguous node shards. Returns per-core (lo, hi) and
    per-core list of graph ids."""
    # graph start offsets (batch sorted)
    counts = np.bincount(batch, minlength=NUM_GRAPHS)
    starts = np.concatenate([[0], np.cumsum(counts)])  # [G+1]
    bounds = [0]
    for k in range(1, NCORES):
        target = k * NUM_NODES // NCORES
        g = int(np.argmin(np.abs(starts - target)))
        bounds.append(int(starts[g]))
    bounds.append(NUM_NODES)
    node_bounds = np.array(bounds)
    graph_bounds = [int(np.searchsorted(starts, b)) for b in bounds]
    return node_bounds, starts, graph_bounds


def _build_edge_blocks(dst_local, src_remap, n_blocks, pad_src, pad_dst):
    """Arrange edges into duplicate-free 128-slot blocks.

    Within-dst rank r -> edges of rank r form a dup-free run; pad each run to
    a multiple of 128. Returns (gidx int32 [n_blocks*128], sidx int32 same)
    where gidx = src row to gather, sidx = dst row to scatter-add.
    """
    order = np.argsort(dst_local, kind="stable")
    ds = dst_local[order]
    ss = src_remap[order]
    # rank within each dst group
    first = np.concatenate([[True], ds[1:] != ds[:-1]])
    idx = np.arange(ds.size)
    start_of_group = np.maximum.accumulate(np.where(first, idx, 0))
    rank = idx - start_of_group
    # order by (rank, dst): stable sort by rank (ds already sorted within)
    order2 = np.argsort(rank, kind="stable")
    ds2, ss2, r2 = ds[order2], ss[order2], rank[order2]
    # pad each rank-run to multiple of 128
    g_parts, s_parts = [], []
    for r in range(int(r2.max()) + 1 if r2.size else 0):
        m = r2 == r
        cnt = int(m.sum())
        if cnt == 0:
            continue
        padded = ((cnt + P - 1) // P) * P
        g = np.full(padded, pad_src, np.int32)
        s = np.full(padded, pad_dst, np.int32)
        g[:cnt] = ss2[m]
        s[:cnt] = ds2[m]
        g_parts.append(g)
        s_parts.append(s)
    gidx = np.concatenate(g_parts) if g_parts else np.zeros(0, np.int32)
    sidx = np.concatenate(s_parts) if s_parts else np.zeros(0, np.int32)
    need = n_blocks * P
    assert gidx.size <= need, (gidx.size, need)
    gpad = np.full(need - gidx.size, pad_src, np.int32)
    spad = np.full(need - sidx.size, pad_dst, np.int32)
    return np.concatenate([gidx, gpad]), np.concatenate([sidx, spad])


def _build_program(n_shard, n_chunks, n_blocks):
    """Bass program; identical across cores. n_shard = N_S (x128),
    n_chunks = n_shard//128, n_blocks = indirect call count per layer."""
    nc = bacc.Bacc("TRN2", target_bir_lowering=False, debug=False,
                   num_devices=NCORES, detect_race_conditions=False)

    N_S = n_shard
    NFULL = NCORES * N_S

    # ---------------- inputs ----------------
    xk = nc.dram_tensor("xk", [N_S, IN_DIM], dt.float32, kind="ExternalInput")
    W1 = nc.dram_tensor("W1", [IN_DIM, HID_DIM], dt.float32, kind="ExternalInput")
    W2 = nc.dram_tensor("W2", [HID_DIM, EMB_DIM], dt.float32, kind="ExternalInput")
    b1r = nc.dram_tensor("b1r", [P, HID_DIM], dt.float32, kind="ExternalInput")
    b2r = nc.dram_tensor("b2r", [P, EMB_DIM], dt.float32, kind="ExternalInput")
    dinvk = nc.dram_tensor("dinvk", [P, n_chunks], dt.float32, kind="ExternalInput")
    ident = nc.dram_tensor("ident", [P, P], dt.float32, kind="ExternalInput")
    # GraphNorm/pool params, replicated per graph-slot row
    gnp1 = nc.dram_tensor("gnp1", [GMAX, 3 * HID_DIM], dt.float32,
                          kind="ExternalInput")  # [alpha|weight|bias]
    gnp2 = nc.dram_tensor("gnp2", [GMAX, 3 * EMB_DIM], dt.float32,
                          kind="ExternalInput")
    cntinv = nc.dram_tensor("cntinv", [GMAX, 1], dt.float32, kind="ExternalInput")
    S_in = nc.dram_tensor("S_in", [n_chunks, P, GMAX], dt.float32,
                          kind="ExternalInput")
    ST_in = nc.dram_tensor("ST_in", [n_chunks, GMAX, P], dt.float32,
                           kind="ExternalInput")
    gidx1 = nc.dram_tensor("gidx1", [P, n_blocks], dt.int32, kind="ExternalInput")
    sidx1 = nc.dram_tensor("sidx1", [P, n_blocks], dt.int32, kind="ExternalInput")
    gidx2 = nc.dram_tensor("gidx2", [P, n_blocks], dt.int32, kind="ExternalInput")
    sidx2 = nc.dram_tensor("sidx2", [P, n_blocks], dt.int32, kind="ExternalInput")

    pool_out = nc.dram_tensor("pool_out", [GMAX, EMB_DIM], dt.float32,
                              kind="ExternalOutput")

    # ---------------- internal DRAM ----------------
    hp_loc = nc.dram_tensor("hp_loc", [N_S, HID_DIM], dt.float32)
    hp_full = nc.dram_tensor("hp_full", [NFULL, HID_DIM], dt.float32,
                             addr_space="Shared")
    hp2_loc = nc.dram_tensor("hp2_loc", [N_S, EMB_DIM], dt.float32)
    hp2_full = nc.dram_tensor("hp2_full", [NFULL, EMB_DIM], dt.float32,
                              addr_space="Shared")
    # aggregation regions (scatter targets); extra 128 dummy rows
    agg1 = nc.dram_tensor("agg1", [N_S + P, HID_DIM], dt.float32)
    agg2 = nc.dram_tensor("agg2", [N_S + P, EMB_DIM], dt.float32)

    F = HID_DIM  # 64 (== EMB_DIM)

    with tile.TileContext(nc) as tc:
        with (
            tc.tile_pool(name="const", bufs=1) as cpool,
            tc.tile_pool(name="work", bufs=3) as wpool,
            tc.tile_pool(name="msg", bufs=8) as mpool,
            tc.tile_pool(name="ps", bufs=2, space="PSUM") as pspool,
            tc.tile_pool(name="psacc", bufs=1, space="PSUM") as paccpool,
            tc.tile_pool(name="ps1", bufs=2, space="PSUM") as pspool1,
        )    :
            idt = cpool.tile([P, P], dt.float32)
            nc.sync.dma_start(idt[:], ident[:])
            w1t = cpool.tile([IN_DIM, HID_DIM], dt.float32)
            nc.sync.dma_start(w1t[:], W1[:])
            w2t = cpool.tile([HID_DIM, EMB_DIM], dt.float32)
            nc.sync.dma_start(w2t[:], W2[:])
            dinvt = cpool.tile([P, n_chunks], dt.float32)
            nc.sync.dma_start(dinvt[:], dinvk[:])
            b1t = cpool.tile([P, HID_DIM], dt.float32)
            nc.sync.dma_start(b1t[:], b1r[:])
            b2t = cpool.tile([P, EMB_DIM], dt.float32)
            nc.sync.dma_start(b2t[:], b2r[:])
            St = cpool.tile([P, n_chunks, GMAX], dt.float32)
            nc.sync.dma_start(St[:], S_in.ap().rearrange("c p g -> p c g"))
            STt = cpool.tile([GMAX, n_chunks, P], dt.float32)
            nc.sync.dma_start(STt[:], ST_in.ap().rearrange("c g p -> g c p"))
            gnp1t = cpool.tile([GMAX, 3 * HID_DIM], dt.float32)
            nc.sync.dma_start(gnp1t[:], gnp1[:])
            gnp2t = cpool.tile([GMAX, 3 * EMB_DIM], dt.float32)
            nc.sync.dma_start(gnp2t[:], gnp2[:])
            cit = cpool.tile([GMAX, 1], dt.float32)
            nc.sync.dma_start(cit[:], cntinv[:])

            # resident x2 (layer outputs, node-major chunks)
            x2t = cpool.tile([P, n_chunks, F], dt.float32)

            def dense_layer(src_chunks_ap, in_dim, wt, hp_dst):
                """hp_dst[c*128+p] = dinv * (x @ W) for each chunk."""
                for c in range(n_chunks):
                    src = src_chunks_ap(c)
                    if src.space == bass.MemorySpace.DRAM:
                        xc = wpool.tile([P, in_dim], dt.float32, tag="xc")
                        nc.sync.dma_start(xc[:], src)
                        src = xc[:]
                    xt_ps = pspool.tile([in_dim, P], dt.float32, tag="xtp")
                    nc.tensor.transpose(out=xt_ps[:], in_=src, identity=idt[:])
                    xts = wpool.tile([in_dim, P], dt.float32, tag="xts")
                    nc.vector.tensor_copy(xts[:], xt_ps[:])
                    h_ps = pspool1.tile([P, F], dt.float32, tag="hps")
                    nc.tensor.matmul(h_ps[:], lhsT=xts[:], rhs=wt[:],
                                     start=True, stop=True)
                    hc = wpool.tile([P, F], dt.float32, tag="hc")
                    nc.vector.tensor_tensor(
                        out=hc[:], in0=h_ps[:],
                        in1=dinvt[:, c:c + 1].to_broadcast([P, F]),
                        op=mybir.AluOpType.mult)
                    nc.sync.dma_start(hp_dst[c * P:(c + 1) * P, :], hc[:])

            def edge_phase(table_full, gidx, sidx, agg, hp_local):
                # init agg[0:N_S] = hp_local (self-loop term), zero dummy rows
                for c in range(n_chunks):
                    t = wpool.tile([P, F], dt.float32, tag="cp")
                    nc.sync.dma_start(t[:], hp_local[c * P:(c + 1) * P, :])
                    nc.sync.dma_start(agg[c * P:(c + 1) * P, :], t[:])
                zt = wpool.tile([P, F], dt.float32, tag="cp")
                nc.vector.memset(zt[:], 0.0)
                nc.sync.dma_start(agg[N_S:N_S + P, :], zt[:])

                gt = cpool.tile([P, n_blocks], dt.int32, tag=f"g{agg.name}")
                nc.sync.dma_start(gt[:], gidx[:])
                st = cpool.tile([P, n_blocks], dt.int32, tag=f"s{agg.name}")
                nc.sync.dma_start(st[:], sidx[:])
                for b in range(n_blocks):
                    msg = mpool.tile([P, F], dt.float32, tag="msg")
                    nc.gpsimd.indirect_dma_start(
                        out=msg[:], out_offset=None,
                        in_=table_full[:],
                        in_offset=bass.IndirectOffsetOnAxis(
                            ap=gt[:, b:b + 1], axis=0))
                    nc.gpsimd.indirect_dma_start(
                        out=agg[:],
                        out_offset=bass.IndirectOffsetOnAxis(
                            ap=st[:, b:b + 1], axis=0),
                        in_=msg[:], in_offset=None,
                        compute_op=mybir.AluOpType.add)

            def post_layer(agg, bt, gnpt, fdim, out_tile):
                """out_tile[:, c, :] = relu(GN(dinv*agg + b)) ; also returns
                stats psum for later reuse. GN per graph via S matmuls."""
                # pass 1: y = dinv*agg + b (chunk-wise, keep resident), stats
                yt = out_tile
                st_ps = paccpool.tile([GMAX, 2 * fdim], dt.float32, tag="stats")
                for c in range(n_chunks):
                    t = wpool.tile([P, fdim], dt.float32, tag="pl1")
                    nc.sync.dma_start(t[:], agg[c * P:(c + 1) * P, :])
                    nc.vector.tensor_tensor(
                        out=t[:], in0=t[:],
                        in1=dinvt[:, c:c + 1].to_broadcast([P, fdim]),
                        op=mybir.AluOpType.mult)
                    nc.vector.tensor_tensor(
                        out=yt[:, c, :], in0=t[:], in1=bt[:],
                        op=mybir.AluOpType.add)
                    sq = wpool.tile([P, 2 * fdim], dt.float32, tag="sq")
                    nc.vector.tensor_copy(sq[:, 0:fdim], yt[:, c, :])
                    nc.vector.tensor_tensor(
                        out=sq[:, fdim:2 * fdim], in0=yt[:, c, :],
                        in1=yt[:, c, :], op=mybir.AluOpType.mult)
                    nc.tensor.matmul(st_ps[:], lhsT=St[:, c, :], rhs=sq[:],
                                     start=(c == 0), stop=(c == n_chunks - 1))
                # stats -> A, B   (alpha|weight|bias in gnpt)
                stats = wpool.tile([GMAX, 2 * fdim], dt.float32, tag="stf")
                nc.vector.tensor_tensor(
                    out=stats[:], in0=st_ps[:],
                    in1=cit[:, 0:1].to_broadcast([GMAX, 2 * fdim]),
                    op=mybir.AluOpType.mult)  # [mean | E[x^2]]
                mean = stats[:, 0:fdim]
                ex2 = stats[:, fdim:2 * fdim]
                alpha = gnpt[:, 0:fdim]
                weight = gnpt[:, fdim:2 * fdim]
                bias = gnpt[:, 2 * fdim:3 * fdim]
                am = wpool.tile([GMAX, fdim], dt.float32, tag="am")
                nc.vector.tensor_tensor(out=am[:], in0=alpha, in1=mean,
                                        op=mybir.AluOpType.mult)  # alpha*m
                var = wpool.tile([GMAX, fdim], dt.float32, tag="var")
                # var = E[x^2] - 2*am*m + am^2 = E[x^2] + am*(am - 2m)
                t2 = wpool.tile([GMAX, fdim], dt.float32, tag="t2")
                nc.vector.tensor_scalar(out=t2[:], in0=mean, scalar1=-2.0,
                                        scalar2=None,
                                        op0=mybir.AluOpType.mult)
                nc.vector.tensor_tensor(out=t2[:], in0=t2[:], in1=am[:],
                                        op=mybir.AluOpType.add)
                nc.vector.tensor_tensor(out=t2[:], in0=t2[:], in1=am[:],
                                        op=mybir.AluOpType.mult)
                nc.vector.tensor_tensor(out=var[:], in0=ex2, in1=t2[:],
                                        op=mybir.AluOpType.add)
                istd = wpool.tile([GMAX, fdim], dt.float32, tag="istd")
                nc.vector.tensor_scalar(out=istd[:], in0=var[:], scalar1=EPS,
                                        scalar2=None, op0=mybir.AluOpType.add)
                nc.scalar.activation(istd[:], istd[:],
                                     mybir.ActivationFunctionType.Sqrt)
                nc.vector.reciprocal(istd[:], istd[:])
                A = wpool.tile([GMAX, fdim], dt.float32, tag="A")
                nc.vector.tensor_tensor(out=A[:], in0=weight, in1=istd[:],
                                        op=mybir.AluOpType.mult)
                B = wpool.tile([GMAX, fdim], dt.float32, tag="B")
                nc.vector.tensor_tensor(out=B[:], in0=A[:], in1=am[:],
                                        op=mybir.AluOpType.mult)
                nc.vector.tensor_scalar(out=B[:], in0=B[:], scalar1=-1.0,
                                        scalar2=None, op0=mybir.AluOpType.mult)
                nc.vector.tensor_tensor(out=B[:], in0=B[:], in1=bias,
                                        op=mybir.AluOpType.add)
                AB = wpool.tile([GMAX, 2 * fdim], dt.float32, tag="AB")
                nc.vector.tensor_copy(AB[:, 0:fdim], A[:])
                nc.vector.tensor_copy(AB[:, fdim:2 * fdim], B[:])
                # pass 2: y = relu(y*Ae + Be) per chunk
                for c in range(n_chunks):
                    ab_ps = pspool.tile([P, 2 * fdim], dt.float32, tag="abps")
                    nc.tensor.matmul(ab_ps[:], lhsT=STt[:, c, :], rhs=AB[:],
                                     start=True, stop=True)
                    nc.vector.tensor_tensor(
                        out=yt[:, c, :], in0=yt[:, c, :],
                        in1=ab_ps[:, 0:fdim], op=mybir.AluOpType.mult)
                    nc.vector.tensor_tensor(
                        out=yt[:, c, :], in0=yt[:, c, :],
                        in1=ab_ps[:, fdim:2 * fdim], op=mybir.AluOpType.add)
                    nc.scalar.activation(yt[:, c, :], yt[:, c, :],
                                         mybir.ActivationFunctionType.Relu)

            # ---------------- layer 1 ----------------
            dense_layer(lambda c: xk[c * P:(c + 1) * P, :], IN_DIM, w1t, hp_loc)
            nc.gpsimd.collective_compute(
                "AllGather", mybir.AluOpType.bypass,
                replica_groups=[list(range(NCORES))],
                ins=[hp_loc.ap()], outs=[hp_full.ap()])
            edge_phase(hp_full, gidx1, sidx1, agg1, hp_loc)
            post_layer(agg1, b1t, gnp1t, HID_DIM, x2t)

            # ---------------- layer 2 ----------------
            dense_layer(lambda c: x2t[:, c, :], HID_DIM, w2t, hp2_loc)
            nc.gpsimd.collective_compute(
                "AllGather", mybir.AluOpType.bypass,
                replica_groups=[list(range(NCORES))],
                ins=[hp2_loc.ap()], outs=[hp2_full.ap()])
            edge_phase(hp2_full, gidx2, sidx2, agg2, hp2_loc)
            h2t = cpool.tile([P, n_chunks, F], dt.float32, tag="h2t")
            post_layer(agg2, b2t, gnp2t, EMB_DIM, h2t)

            # ---------------- pooling ----------------
            pl_ps = paccpool.tile([GMAX, EMB_DIM], dt.float32, tag="plps")
            for c in range(n_chunks):
                nc.tensor.matmul(pl_ps[:], lhsT=St[:, c, :], rhs=h2t[:, c, :],
                                 start=(c == 0), stop=(c == n_chunks - 1))
            plt = wpool.tile([GMAX, EMB_DIM], dt.float32, tag="plt")
            nc.vector.tensor_tensor(
                out=plt[:], in0=pl_ps[:],
                in1=cit[:, 0:1].to_broadcast([GMAX, EMB_DIM]),
                op=mybir.AluOpType.mult)
            nc.sync.dma_start(pool_out[:], plt[:])

    nc.compile()
    return nc


def kernel(x, edge_index, batch, W1, b1, alpha1, weight1, bias1,
           W2, b2, alpha2, weight2, bias2):
    x = np.asarray(x, np.float32)
    edge_index = np.asarray(edge_index, np.int32)
    batch = np.asarray(batch, np.int32)

    src, dst = edge_index[0].astype(np.int64), edge_index[1].astype(np.int64)
    deg = np.bincount(dst, minlength=NUM_NODES).astype(np.float32) + 1.0
    dinv = 1.0 / np.sqrt(deg)

    node_bounds, gstarts, graph_bounds = _shard_plan(batch)
    n_shard = int(np.max(node_bounds[1:] - node_bounds[:-1]))
    n_shard = ((n_shard + P - 1) // P) * P
    n_chunks = n_shard // P

    # remap node -> (core, local)
    core_of = np.searchsorted(node_bounds, np.arange(NUM_NODES), side="right") - 1
    local = np.arange(NUM_NODES) - node_bounds[core_of]
    remap = core_of * n_shard + local   # row in hp_full

    ecore = core_of[dst]
    # per-core block counts (must be equal across cores -> pad to max)
    per_core = []
    max_blocks = 0
    for k in range(NCORES):
        m = ecore == k
        sl = src[m]
        dl = dst[m] - node_bounds[k]
        # compute blocks needed: sum over ranks of ceil(cnt_r/128)
        order = np.argsort(dl, kind="stable")
        dls = dl[order]
        first = np.concatenate([[True], dls[1:] != dls[:-1]])
        idxa = np.arange(dls.size)
        sog = np.maximum.accumulate(np.where(first, idxa, 0))
        rank = idxa - sog
        nb = 0
        for r in range(int(rank.max()) + 1 if rank.size else 0):
            cnt = int((rank == r).sum())
            nb += (cnt + P - 1) // P
        max_blocks = max(max_blocks, nb)
        per_core.append((sl, dl))
    n_blocks = max_blocks

    key = (n_shard, n_chunks, n_blocks)
    if key not in _CACHE:
        _CACHE[key] = _build_program(n_shard, n_chunks, n_blocks)
    nc = _CACHE[key]

    ident = np.eye(P, dtype=np.float32)

    in_maps = []
    pool_maps = []
    for k in range(NCORES):
        lo, hi = int(node_bounds[k]), int(node_bounds[k + 1])
        nk = hi - lo
        xk = np.zeros((n_shard, IN_DIM), np.float32)
        xk[:nk] = x[lo:hi]
        dv = np.zeros(n_shard, np.float32)
        dv[:nk] = dinv[lo:hi]
        dinvk = dv.reshape(n_chunks, P).T.copy()   # [P, n_chunks]

        sl, dl = per_core[k]
        srm = remap[sl].astype(np.int32)
        gidx, sidx = _build_edge_blocks(dl.astype(np.int32), srm, n_blocks,
                                        pad_src=np.int32(k * n_shard + nk if nk < n_shard else k * n_shard),
                                        pad_dst=np.int32(n_shard))
        # layout [P, n_blocks]: block b slot p at [p, b]
        g1 = gidx.reshape(n_blocks, P).T.copy()
        s1 = sidx.reshape(n_blocks, P).T.copy()

        glo, ghi = graph_bounds[k], graph_bounds[k + 1]
        ngr = ghi - glo
        assert ngr <= GMAX, ngr
        # S [n_chunks, P, GMAX] one-hot graph membership for local nodes
        gb = np.zeros(n_shard, np.int64)
        gb[:nk] = batch[lo:hi] - glo
        S = np.zeros((n_shard, GMAX), np.float32)
        S[np.arange(nk), gb[:nk]] = 1.0
        S3 = S.reshape(n_chunks, P, GMAX)
        ST3 = np.ascontiguousarray(S3.transpose(0, 2, 1))
        cnts = np.bincount(gb[:nk], minlength=GMAX).astype(np.float32)
        cntinv = (1.0 / np.maximum(cnts, 1.0)).reshape(GMAX, 1).astype(np.float32)

        gnp1 = np.concatenate([
            np.tile(alpha1, (GMAX, 1)), np.tile(weight1, (GMAX, 1)),
            np.tile(bias1, (GMAX, 1))], axis=1).astype(np.float32)
        gnp2 = np.concatenate([
            np.tile(alpha2, (GMAX, 1)), np.tile(weight2, (GMAX, 1)),
            np.tile(bias2, (GMAX, 1))], axis=1).astype(np.float32)

        in_maps.append({
            "xk": xk, "W1": np.asarray(W1, np.float32),
            "W2": np.asarray(W2, np.float32),
            "b1r": np.tile(np.asarray(b1, np.float32), (P, 1)),
            "b2r": np.tile(np.asarray(b2, np.float32), (P, 1)),
            "dinvk": np.ascontiguousarray(dinvk), "ident": ident,
            "gnp1": gnp1, "gnp2": gnp2, "cntinv": cntinv,
            "S_in": np.ascontiguousarray(S3), "ST_in": ST3,
            "gidx1": np.ascontiguousarray(g1), "sidx1": np.ascontiguousarray(s1),
            "gidx2": np.ascontiguousarray(g1), "sidx2": np.ascontiguousarray(s1),
        })
        pool_maps.append((glo, ghi))

    res = run_bass_kernel_spmd(nc, in_maps, list(range(NCORES)))

    out = np.zeros((NUM_GRAPHS, EMB_DIM), np.float32)
    for k in range(NCORES):
        glo, ghi = pool_maps[k]
        out[glo:ghi] = np.asarray(res.results[k]["pool_out"])[:ghi - glo]
    return out


# revision 6
# speedup vs baseline: 2.2026x; 2.2026x over previous
"""GCN encoder (2x GCNConv + GraphNorm + ReLU + mean-pool) on 8 trn2 cores.

Strategy: graph-aligned node sharding across 8 cores (batch is sorted, so
each core owns a contiguous run of whole graphs -> GraphNorm and pooling are
fully shard-local). Edges are assigned to the core that owns their dst node.
Each core:
  - computes hp = dinv * (x @ W1) for its shard (PE),
  - AllGathers hp so every core holds the full node table,
  - aggregates messages with indirect-DMA gathers (128 rows/call) and
    duplicate-free indirect-DMA scatter-adds into its dst shard,
  - applies symmetric-norm scaling, bias, GraphNorm, ReLU (DVE/ACT/PE),
  - repeats for layer 2, then computes per-graph mean pooling (PE).
Host side: numpy preprocessing (degrees, sharding, edge blocking) + final
assembly of the [128, 64] output.
"""
import numpy as np

import concourse.bass as bass
import concourse.bacc as bacc
import concourse.mybir as mybir
import concourse.tile as tile
from concourse.bass_utils import run_bass_kernel_spmd

dt = mybir.dt

NCORES = 8
NUM_NODES = 100000
NUM_EDGES = 1600000
NUM_GRAPHS = 128
IN_DIM, HID_DIM, EMB_DIM = 128, 64, 64
EPS = 1e-5
GMAX = 32          # max graphs per core (128/8 = 16 avg)
P = 128

_CACHE = {}


def _shard_plan(batch):
    """Graph-aligned contiguous node shards. Returns per-core (lo, hi) and
    per-core list of graph ids."""
    # graph start offsets (batch sorted)
    counts = np.bincount(batch, minlength=NUM_GRAPHS)
    starts = np.concatenate([[0], np.cumsum(counts)])  # [G+1]
    bounds = [0]
    for k in range(1, NCORES):
        target = k * NUM_NODES // NCORES
        g = int(np.argmin(np.abs(starts - target)))
        bounds.append(int(starts[g]))
    bounds.append(NUM_NODES)
    node_bounds = np.array(bounds)
    graph_bounds = [int(np.searchsorted(starts, b)) for b in bounds]
    return node_bounds, starts, graph_bounds


def _window_schedule(dst_local, src_remap, n_chunks, c_fix, pad_src):
    """dst-sorted edges packed into per-128-dst-window chunk grids.

    Returns gidx int32 [n_chunks*c_fix*128] (gather row, pad -> pad_src
    zero row) and dstv f32 same shape (window-local dst in [0,128), pads 0).
    Window w occupies chunk slots [w*c_fix, (w+1)*c_fix).
    """
    order = np.argsort(dst_local, kind="stable")
    ds = dst_local[order]
    ss = src_remap[order]
    win = ds // P
    gidx = np.full(n_chunks * c_fix * P, pad_src, np.int64)
    dstv = np.zeros(n_chunks * c_fix * P, np.float32)
    for w in range(n_chunks):
        m = win == w
        cnt = int(m.sum())
        assert cnt <= c_fix * P, (w, cnt, c_fix)
        base = w * c_fix * P
        gidx[base:base + cnt] = ss[m]
        dstv[base:base + cnt] = (ds[m] - w * P).astype(np.float32)
    return gidx.astype(np.int32), dstv


def _build_program(n_shard, n_chunks, n_blocks):
    """Bass program; identical across cores. n_shard = N_S (x128),
    n_chunks = n_shard//128, n_blocks = indirect call count per layer."""
    nc = bacc.Bacc("TRN2", target_bir_lowering=False, debug=False,
                   num_devices=NCORES, detect_race_conditions=False)

    N_S = n_shard
    NFULL = NCORES * N_S

    # ---------------- inputs ----------------
    xk = nc.dram_tensor("xk", [N_S, IN_DIM], dt.float32, kind="ExternalInput")
    W1 = nc.dram_tensor("W1", [IN_DIM, HID_DIM], dt.float32, kind="ExternalInput")
    W2 = nc.dram_tensor("W2", [HID_DIM, EMB_DIM], dt.float32, kind="ExternalInput")
    b1r = nc.dram_tensor("b1r", [P, HID_DIM], dt.float32, kind="ExternalInput")
    b2r = nc.dram_tensor("b2r", [P, EMB_DIM], dt.float32, kind="ExternalInput")
    dinvk = nc.dram_tensor("dinvk", [P, n_chunks], dt.float32, kind="ExternalInput")
    ident = nc.dram_tensor("ident", [P, P], dt.float32, kind="ExternalInput")
    # GraphNorm/pool params, replicated per graph-slot row
    gnp1 = nc.dram_tensor("gnp1", [GMAX, 3 * HID_DIM], dt.float32,
                          kind="ExternalInput")  # [alpha|weight|bias]
    gnp2 = nc.dram_tensor("gnp2", [GMAX, 3 * EMB_DIM], dt.float32,
                          kind="ExternalInput")
    cntinv = nc.dram_tensor("cntinv", [GMAX, 1], dt.float32, kind="ExternalInput")
    S_in = nc.dram_tensor("S_in", [n_chunks, P, GMAX], dt.float32,
                          kind="ExternalInput")
    ST_in = nc.dram_tensor("ST_in", [n_chunks, GMAX, P], dt.float32,
                           kind="ExternalInput")
    gidx1 = nc.dram_tensor("gidx1", [P, n_blocks], dt.int32, kind="ExternalInput")
    dstv1 = nc.dram_tensor("dstv1", [P, n_blocks], dt.float32, kind="ExternalInput")
    iota_in = nc.dram_tensor("iota_in", [P, P], dt.float32, kind="ExternalInput")

    pool_out = nc.dram_tensor("pool_out", [GMAX, EMB_DIM], dt.float32,
                              kind="ExternalOutput")

    # ---------------- internal DRAM ----------------
    hp_loc = nc.dram_tensor("hp_loc", [N_S, HID_DIM], dt.float32)
    hp_full = nc.dram_tensor("hp_full", [NFULL, HID_DIM], dt.float32,
                             addr_space="Shared")
    hp2_loc = nc.dram_tensor("hp2_loc", [N_S, EMB_DIM], dt.float32)
    hp2_full = nc.dram_tensor("hp2_full", [NFULL, EMB_DIM], dt.float32,
                              addr_space="Shared")
    agg1 = nc.dram_tensor("agg1", [N_S, HID_DIM], dt.float32)
    agg2 = nc.dram_tensor("agg2", [N_S, EMB_DIM], dt.float32)

    F = HID_DIM  # 64 (== EMB_DIM)

    with tile.TileContext(nc) as tc:
        with (
            tc.tile_pool(name="const", bufs=1) as cpool,
            tc.tile_pool(name="work", bufs=3) as wpool,
            tc.tile_pool(name="msg", bufs=8) as mpool,
            tc.tile_pool(name="ps", bufs=1, space="PSUM") as pspool,
            tc.tile_pool(name="psacc", bufs=1, space="PSUM") as paccpool,
            tc.tile_pool(name="ps1", bufs=2, space="PSUM") as pspool1,
        )    :
            idt = cpool.tile([P, P], dt.float32)
            nc.sync.dma_start(idt[:], ident[:])
            w1t = cpool.tile([IN_DIM, HID_DIM], dt.float32)
            nc.sync.dma_start(w1t[:], W1[:])
            w2t = cpool.tile([HID_DIM, EMB_DIM], dt.float32)
            nc.sync.dma_start(w2t[:], W2[:])
            dinvt = cpool.tile([P, n_chunks], dt.float32)
            nc.sync.dma_start(dinvt[:], dinvk[:])
            b1t = cpool.tile([P, HID_DIM], dt.float32)
            nc.sync.dma_start(b1t[:], b1r[:])
            b2t = cpool.tile([P, EMB_DIM], dt.float32)
            nc.sync.dma_start(b2t[:], b2r[:])
            St = cpool.tile([P, n_chunks, GMAX], dt.float32)
            nc.sync.dma_start(St[:], S_in.ap().rearrange("c p g -> p c g"))
            STt = cpool.tile([GMAX, n_chunks, P], dt.float32)
            nc.sync.dma_start(STt[:], ST_in.ap().rearrange("c g p -> g c p"))
            gnp1t = cpool.tile([GMAX, 3 * HID_DIM], dt.float32)
            nc.sync.dma_start(gnp1t[:], gnp1[:])
            gnp2t = cpool.tile([GMAX, 3 * EMB_DIM], dt.float32)
            nc.sync.dma_start(gnp2t[:], gnp2[:])
            cit = cpool.tile([GMAX, 1], dt.float32)
            nc.sync.dma_start(cit[:], cntinv[:])
            iot = cpool.tile([P, P], dt.float32)
            nc.sync.dma_start(iot[:], iota_in[:])
            gt = cpool.tile([P, n_blocks], dt.int32)
            nc.sync.dma_start(gt[:], gidx1[:])
            dvt = cpool.tile([P, n_blocks], dt.float32)
            nc.sync.dma_start(dvt[:], dstv1[:])

            # resident x2 (layer outputs, node-major chunks)
            x2t = cpool.tile([P, n_chunks, F], dt.float32)

            def dense_layer(src_chunks_ap, in_dim, wt, hp_dst):
                """hp_dst[c*128+p] = dinv * (x @ W) for each chunk."""
                for c in range(n_chunks):
                    src = src_chunks_ap(c)
                    if src.space == bass.MemorySpace.DRAM:
                        xc = wpool.tile([P, in_dim], dt.float32, tag="xc")
                        nc.sync.dma_start(xc[:], src)
                        src = xc[:]
                    xt_ps = pspool.tile([in_dim, P], dt.float32, tag="xtp")
                    nc.tensor.transpose(out=xt_ps[:], in_=src, identity=idt[:])
                    xts = wpool.tile([in_dim, P], dt.float32, tag="xts")
                    nc.vector.tensor_copy(xts[:], xt_ps[:])
                    h_ps = pspool1.tile([P, F], dt.float32, tag="hps")
                    nc.tensor.matmul(h_ps[:], lhsT=xts[:], rhs=wt[:],
                                     start=True, stop=True)
                    hc = wpool.tile([P, F], dt.float32, tag="hc")
                    nc.vector.tensor_tensor(
                        out=hc[:], in0=h_ps[:],
                        in1=dinvt[:, c:c + 1].to_broadcast([P, F]),
                        op=mybir.AluOpType.mult)
                    nc.sync.dma_start(hp_dst[c * P:(c + 1) * P, :], hc[:])

            c_fix = n_blocks // n_chunks

            def edge_phase(table_full, agg, hp_local):
                for w in range(n_chunks):
                    ps = pspool1.tile([P, F], dt.float32, tag="aggps")
                    hpw = wpool.tile([P, F], dt.float32, tag="hpw")
                    nc.sync.dma_start(hpw[:], hp_local[w * P:(w + 1) * P, :])
                    nc.tensor.matmul(ps[:], lhsT=idt[:], rhs=hpw[:],
                                     start=True, stop=False)
                    for j in range(c_fix):
                        b = w * c_fix + j
                        msg = mpool.tile([P, F], dt.float32, tag="msg")
                        nc.gpsimd.indirect_dma_start(
                            out=msg[:], out_offset=None,
                            in_=table_full[:],
                            in_offset=bass.IndirectOffsetOnAxis(
                                ap=gt[:, b:b + 1], axis=0))
                        S = mpool.tile([P, P], dt.float32, tag="sel")
                        nc.vector.tensor_tensor(
                            out=S[:],
                            in0=dvt[:, b:b + 1].to_broadcast([P, P]),
                            in1=iot[:], op=mybir.AluOpType.is_equal)
                        nc.tensor.matmul(ps[:], lhsT=S[:], rhs=msg[:],
                                         start=False, stop=(j == c_fix - 1))
                    outw = wpool.tile([P, F], dt.float32, tag="outw")
                    nc.vector.tensor_copy(outw[:], ps[:])
                    nc.sync.dma_start(agg[w * P:(w + 1) * P, :], outw[:])

            def post_layer(agg, bt, gnpt, fdim, out_tile):
                """out_tile[:, c, :] = relu(GN(dinv*agg + b)) ; also returns
                stats psum for later reuse. GN per graph via S matmuls."""
                # pass 1: y = dinv*agg + b (chunk-wise, keep resident), stats
                yt = out_tile
                st_ps = paccpool.tile([GMAX, 2 * fdim], dt.float32, tag="stats")
                for c in range(n_chunks):
                    t = wpool.tile([P, fdim], dt.float32, tag="pl1")
                    nc.sync.dma_start(t[:], agg[c * P:(c + 1) * P, :])
                    nc.vector.tensor_tensor(
                        out=t[:], in0=t[:],
                        in1=dinvt[:, c:c + 1].to_broadcast([P, fdim]),
                        op=mybir.AluOpType.mult)
                    nc.vector.tensor_tensor(
                        out=yt[:, c, :], in0=t[:], in1=bt[:],
                        op=mybir.AluOpType.add)
                    sq = wpool.tile([P, 2 * fdim], dt.float32, tag="sq")
                    nc.vector.tensor_copy(sq[:, 0:fdim], yt[:, c, :])
                    nc.vector.tensor_tensor(
                        out=sq[:, fdim:2 * fdim], in0=yt[:, c, :],
                        in1=yt[:, c, :], op=mybir.AluOpType.mult)
                    nc.tensor.matmul(st_ps[:], lhsT=St[:, c, :], rhs=sq[:],
                                     start=(c == 0), stop=(c == n_chunks - 1))
                # stats -> A, B   (alpha|weight|bias in gnpt)
                stats = wpool.tile([GMAX, 2 * fdim], dt.float32, tag="stf")
                nc.vector.tensor_tensor(
                    out=stats[:], in0=st_ps[:],
                    in1=cit[:, 0:1].to_broadcast([GMAX, 2 * fdim]),
                    op=mybir.AluOpType.mult)  # [mean | E[x^2]]
                mean = stats[:, 0:fdim]
                ex2 = stats[:, fdim:2 * fdim]
                alpha = gnpt[:, 0:fdim]
                weight = gnpt[:, fdim:2 * fdim]
                bias = gnpt[:, 2 * fdim:3 * fdim]
                am = wpool.tile([GMAX, fdim], dt.float32, tag="am")
                nc.vector.tensor_tensor(out=am[:], in0=alpha, in1=mean,
                                        op=mybir.AluOpType.mult)  # alpha*m
                var = wpool.tile([GMAX, fdim], dt.float32, tag="var")
                # var = E[x^2] - 2*am*m + am^2 = E[x^2] + am*(am - 2m)
                t2 = wpool.tile([GMAX, fdim], dt.float32, tag="t2")
                nc.vector.tensor_scalar(out=t2[:], in0=mean, scalar1=-2.0,
                                        scalar2=None,
                                        op0=mybir.AluOpType.mult)
                nc.vector.tensor_tensor(out=t2[:], in0=t2[:], in1=am[:],
                                        op=mybir.AluOpType.add)
                nc.vector.tensor_tensor(out=t2[:], in0=t2[:], in1=am[:],
                                        op=mybir.AluOpType.mult)
                nc.vector.tensor_tensor(out=var[:], in0=ex2, in1=t2[:],
                                        op=mybir.AluOpType.add)
                istd = wpool.tile([GMAX, fdim], dt.float32, tag="istd")
                nc.vector.tensor_scalar(out=istd[:], in0=var[:], scalar1=EPS,
                                        scalar2=None, op0=mybir.AluOpType.add)
                nc.scalar.activation(istd[:], istd[:],
                                     mybir.ActivationFunctionType.Sqrt)
                nc.vector.reciprocal(istd[:], istd[:])
                A = wpool.tile([GMAX, fdim], dt.float32, tag="A")
                nc.vector.tensor_tensor(out=A[:], in0=weight, in1=istd[:],
                                        op=mybir.AluOpType.mult)
                B = wpool.tile([GMAX, fdim], dt.float32, tag="B")
                nc.vector.tensor_tensor(out=B[:], in0=A[:], in1=am[:],
                                        op=mybir.AluOpType.mult)
                nc.vector.tensor_scalar(out=B[:], in0=B[:], scalar1=-1.0,
                                        scalar2=None, op0=mybir.AluOpType.mult)
                nc.vector.tensor_tensor(out=B[:], in0=B[:], in1=bias,
                                        op=mybir.AluOpType.add)
                AB = wpool.tile([GMAX, 2 * fdim], dt.float32, tag="AB")
                nc.vector.tensor_copy(AB[:, 0:fdim], A[:])
                nc.vector.tensor_copy(AB[:, fdim:2 * fdim], B[:])
                # pass 2: y = relu(y*Ae + Be) per chunk
                for c in range(n_chunks):
                    ab_ps = pspool.tile([P, 2 * fdim], dt.float32, tag="abps")
                    nc.tensor.matmul(ab_ps[:], lhsT=STt[:, c, :], rhs=AB[:],
                                     start=True, stop=True)
                    nc.vector.tensor_tensor(
                        out=yt[:, c, :], in0=yt[:, c, :],
                        in1=ab_ps[:, 0:fdim], op=mybir.AluOpType.mult)
                    nc.vector.tensor_tensor(
                        out=yt[:, c, :], in0=yt[:, c, :],
                        in1=ab_ps[:, fdim:2 * fdim], op=mybir.AluOpType.add)
                    nc.scalar.activation(yt[:, c, :], yt[:, c, :],
                                         mybir.ActivationFunctionType.Relu)

            # ---------------- layer 1 ----------------
            dense_layer(lambda c: xk[c * P:(c + 1) * P, :], IN_DIM, w1t, hp_loc)
            nc.gpsimd.collective_compute(
                "AllGather", mybir.AluOpType.bypass,
                replica_groups=[list(range(NCORES))],
                ins=[hp_loc.ap()], outs=[hp_full.ap()])
            edge_phase(hp_full, agg1, hp_loc)
            post_layer(agg1, b1t, gnp1t, HID_DIM, x2t)

            # ---------------- layer 2 ----------------
            dense_layer(lambda c: x2t[:, c, :], HID_DIM, w2t, hp2_loc)
            nc.gpsimd.collective_compute(
                "AllGather", mybir.AluOpType.bypass,
                replica_groups=[list(range(NCORES))],
                ins=[hp2_loc.ap()], outs=[hp2_full.ap()])
            edge_phase(hp2_full, agg2, hp2_loc)
            h2t = cpool.tile([P, n_chunks, F], dt.float32, tag="h2t")
            post_layer(agg2, b2t, gnp2t, EMB_DIM, h2t)

            # ---------------- pooling ----------------
            pl_ps = paccpool.tile([GMAX, EMB_DIM], dt.float32, tag="plps")
            for c in range(n_chunks):
                nc.tensor.matmul(pl_ps[:], lhsT=St[:, c, :], rhs=h2t[:, c, :],
                                 start=(c == 0), stop=(c == n_chunks - 1))
            plt = wpool.tile([GMAX, EMB_DIM], dt.float32, tag="plt")
            nc.vector.tensor_tensor(
                out=plt[:], in0=pl_ps[:],
                in1=cit[:, 0:1].to_broadcast([GMAX, EMB_DIM]),
                op=mybir.AluOpType.mult)
            nc.sync.dma_start(pool_out[:], plt[:])

    nc.compile()
    return nc


def kernel(x, edge_index, batch, W1, b1, alpha1, weight1, bias1,
           W2, b2, alpha2, weight2, bias2):
    x = np.asarray(x, np.float32)
    edge_index = np.asarray(edge_index, np.int32)
    batch = np.asarray(batch, np.int32)

    src, dst = edge_index[0].astype(np.int64), edge_index[1].astype(np.int64)
    deg = np.bincount(dst, minlength=NUM_NODES).astype(np.float32) + 1.0
    dinv = 1.0 / np.sqrt(deg)

    node_bounds, gstarts, graph_bounds = _shard_plan(batch)
    n_shard = int(np.max(node_bounds[1:] - node_bounds[:-1]))
    # +P guarantees every core has >=1 all-zero pad row (gather target for
    # padding slots in the edge schedule)
    n_shard = ((n_shard + P) // P) * P
    n_chunks = n_shard // P

    # remap node -> (core, local)
    core_of = np.searchsorted(node_bounds, np.arange(NUM_NODES), side="right") - 1
    local = np.arange(NUM_NODES) - node_bounds[core_of]
    remap = core_of * n_shard + local   # row in hp_full

    ecore = core_of[dst]
    per_core = []
    c_fix = 0
    for k in range(NCORES):
        m = ecore == k
        sl = src[m]
        dl = dst[m] - node_bounds[k]
        wcnt = np.bincount(dl // P, minlength=n_chunks)
        c_fix = max(c_fix, int(np.max((wcnt + P - 1) // P)))
        per_core.append((sl, dl))
    n_blocks = n_chunks * c_fix

    key = (n_shard, n_chunks, n_blocks)
    if key not in _CACHE:
        _CACHE[key] = _build_program(n_shard, n_chunks, n_blocks)
    nc = _CACHE[key]

    ident = np.eye(P, dtype=np.float32)

    in_maps = []
    pool_maps = []
    for k in range(NCORES):
        lo, hi = int(node_bounds[k]), int(node_bounds[k + 1])
        nk = hi - lo
        xk = np.zeros((n_shard, IN_DIM), np.float32)
        xk[:nk] = x[lo:hi]
        dv = np.zeros(n_shard, np.float32)
        dv[:nk] = dinv[lo:hi]
        dinvk = dv.reshape(n_chunks, P).T.copy()   # [P, n_chunks]

        sl, dl = per_core[k]
        srm = remap[sl]
        pad_src = k * n_shard + nk if nk < n_shard else k * n_shard
        gidx, dstv = _window_schedule(dl, srm, n_chunks, n_blocks // n_chunks,
                                      pad_src)
        g1 = gidx.reshape(n_blocks, P).T.copy()
        d1 = dstv.reshape(n_blocks, P).T.copy()

        glo, ghi = graph_bounds[k], graph_bounds[k + 1]
        ngr = ghi - glo
        assert ngr <= GMAX, ngr
        # S [n_chunks, P, GMAX] one-hot graph membership for local nodes
        gb = np.zeros(n_shard, np.int64)
        gb[:nk] = batch[lo:hi] - glo
        S = np.zeros((n_shard, GMAX), np.float32)
        S[np.arange(nk), gb[:nk]] = 1.0
        S3 = S.reshape(n_chunks, P, GMAX)
        ST3 = np.ascontiguousarray(S3.transpose(0, 2, 1))
        cnts = np.bincount(gb[:nk], minlength=GMAX).astype(np.float32)
        cntinv = (1.0 / np.maximum(cnts, 1.0)).reshape(GMAX, 1).astype(np.float32)

        gnp1 = np.concatenate([
            np.tile(alpha1, (GMAX, 1)), np.tile(weight1, (GMAX, 1)),
            np.tile(bias1, (GMAX, 1))], axis=1).astype(np.float32)
        gnp2 = np.concatenate([
            np.tile(alpha2, (GMAX, 1)), np.tile(weight2, (GMAX, 1)),
            np.tile(bias2, (GMAX, 1))], axis=1).astype(np.float32)

        in_maps.append({
            "xk": xk, "W1": np.asarray(W1, np.float32),
            "W2": np.asarray(W2, np.float32),
            "b1r": np.tile(np.asarray(b1, np.float32), (P, 1)),
            "b2r": np.tile(np.asarray(b2, np.float32), (P, 1)),
            "dinvk": np.ascontiguousarray(dinvk), "ident": ident,
            "gnp1": gnp1, "gnp2": gnp2, "cntinv": cntinv,
            "S_in": np.ascontiguousarray(S3), "ST_in": ST3,
            "gidx1": np.ascontiguousarray(g1),
            "dstv1": np.ascontiguousarray(d1),
            "iota_in": np.tile(np.arange(P, dtype=np.float32), (P, 1)),
        })
        pool_maps.append((glo, ghi))

    res = run_bass_kernel_spmd(nc, in_maps, list(range(NCORES)))

    out = np.zeros((NUM_GRAPHS, EMB_DIM), np.float32)
    for k in range(NCORES):
        glo, ghi = pool_maps[k]
        out[glo:ghi] = np.asarray(res.results[k]["pool_out"])[:ghi - glo]
    return out


# revision 7
# speedup vs baseline: 2.2055x; 1.0013x over previous
"""GCN encoder (2x GCNConv + GraphNorm + ReLU + mean-pool) on 8 trn2 cores.

Strategy: graph-aligned node sharding across 8 cores (batch is sorted, so
each core owns a contiguous run of whole graphs -> GraphNorm and pooling are
fully shard-local). Edges are assigned to the core that owns their dst node.
Each core:
  - computes hp = dinv * (x @ W1) for its shard (PE),
  - AllGathers hp so every core holds the full node table,
  - aggregates messages with indirect-DMA gathers (128 rows/call) and
    duplicate-free indirect-DMA scatter-adds into its dst shard,
  - applies symmetric-norm scaling, bias, GraphNorm, ReLU (DVE/ACT/PE),
  - repeats for layer 2, then computes per-graph mean pooling (PE).
Host side: numpy preprocessing (degrees, sharding, edge blocking) + final
assembly of the [128, 64] output.
"""
import numpy as np

import concourse.bass as bass
import concourse.bacc as bacc
import concourse.mybir as mybir
import concourse.tile as tile
from concourse.bass_utils import run_bass_kernel_spmd

dt = mybir.dt

NCORES = 8
NUM_NODES = 100000
NUM_EDGES = 1600000
NUM_GRAPHS = 128
IN_DIM, HID_DIM, EMB_DIM = 128, 64, 64
EPS = 1e-5
GMAX = 32          # max graphs per core (128/8 = 16 avg)
P = 128

_CACHE = {}


def _shard_plan(batch):
    """Graph-aligned contiguous node shards. Returns per-core (lo, hi) and
    per-core list of graph ids."""
    # graph start offsets (batch sorted)
    counts = np.bincount(batch, minlength=NUM_GRAPHS)
    starts = np.concatenate([[0], np.cumsum(counts)])  # [G+1]
    bounds = [0]
    for k in range(1, NCORES):
        target = k * NUM_NODES // NCORES
        g = int(np.argmin(np.abs(starts - target)))
        bounds.append(int(starts[g]))
    bounds.append(NUM_NODES)
    node_bounds = np.array(bounds)
    graph_bounds = [int(np.searchsorted(starts, b)) for b in bounds]
    return node_bounds, starts, graph_bounds


def _window_schedule(dst_local, src_remap, n_chunks, c_fix, pad_src):
    """dst-sorted edges packed into per-128-dst-window chunk grids.

    Returns gidx int32 [n_chunks*c_fix*128] (gather row, pad -> pad_src
    zero row) and dstv f32 same shape (window-local dst in [0,128), pads 0).
    Window w occupies chunk slots [w*c_fix, (w+1)*c_fix).
    """
    order = np.argsort(dst_local, kind="stable")
    ds = dst_local[order]
    ss = src_remap[order]
    win = ds // P
    gidx = np.full(n_chunks * c_fix * P, pad_src, np.int64)
    dstv = np.zeros(n_chunks * c_fix * P, np.float32)
    for w in range(n_chunks):
        m = win == w
        cnt = int(m.sum())
        assert cnt <= c_fix * P, (w, cnt, c_fix)
        base = w * c_fix * P
        gidx[base:base + cnt] = ss[m]
        dstv[base:base + cnt] = (ds[m] - w * P).astype(np.float32)
    return gidx.astype(np.int32), dstv


def _build_program(n_shard, n_chunks, n_blocks):
    """Bass program; identical across cores. n_shard = N_S (x128),
    n_chunks = n_shard//128, n_blocks = indirect call count per layer."""
    nc = bacc.Bacc("TRN2", target_bir_lowering=False, debug=False,
                   num_devices=NCORES, detect_race_conditions=False)

    N_S = n_shard
    NFULL = NCORES * N_S

    # ---------------- inputs ----------------
    xk = nc.dram_tensor("xk", [N_S, IN_DIM], dt.float32, kind="ExternalInput")
    W1 = nc.dram_tensor("W1", [IN_DIM, HID_DIM], dt.float32, kind="ExternalInput")
    W2 = nc.dram_tensor("W2", [HID_DIM, EMB_DIM], dt.float32, kind="ExternalInput")
    b1r = nc.dram_tensor("b1r", [P, HID_DIM], dt.float32, kind="ExternalInput")
    b2r = nc.dram_tensor("b2r", [P, EMB_DIM], dt.float32, kind="ExternalInput")
    dinvk = nc.dram_tensor("dinvk", [P, n_chunks], dt.float32, kind="ExternalInput")
    ident = nc.dram_tensor("ident", [P, P], dt.float32, kind="ExternalInput")
    # GraphNorm/pool params, replicated per graph-slot row
    gnp1 = nc.dram_tensor("gnp1", [GMAX, 3 * HID_DIM], dt.float32,
                          kind="ExternalInput")  # [alpha|weight|bias]
    gnp2 = nc.dram_tensor("gnp2", [GMAX, 3 * EMB_DIM], dt.float32,
                          kind="ExternalInput")
    cntinv = nc.dram_tensor("cntinv", [GMAX, 1], dt.float32, kind="ExternalInput")
    S_in = nc.dram_tensor("S_in", [n_chunks, P, GMAX], dt.float32,
                          kind="ExternalInput")
    ST_in = nc.dram_tensor("ST_in", [n_chunks, GMAX, P], dt.float32,
                           kind="ExternalInput")
    gidx1 = nc.dram_tensor("gidx1", [P, n_blocks], dt.int32, kind="ExternalInput")
    dstv1 = nc.dram_tensor("dstv1", [P, n_blocks], dt.float32, kind="ExternalInput")
    iota_in = nc.dram_tensor("iota_in", [P, P], dt.float32, kind="ExternalInput")

    pool_out = nc.dram_tensor("pool_out", [GMAX, EMB_DIM], dt.float32,
                              kind="ExternalOutput")

    # ---------------- internal DRAM ----------------
    hp_loc = nc.dram_tensor("hp_loc", [N_S, HID_DIM], dt.float32)
    hp_full = nc.dram_tensor("hp_full", [NFULL, HID_DIM], dt.float32,
                             addr_space="Shared")
    hp2_loc = nc.dram_tensor("hp2_loc", [N_S, EMB_DIM], dt.float32)
    hp2_full = nc.dram_tensor("hp2_full", [NFULL, EMB_DIM], dt.float32,
                              addr_space="Shared")
    agg1 = nc.dram_tensor("agg1", [N_S, HID_DIM], dt.float32)
    agg2 = nc.dram_tensor("agg2", [N_S, EMB_DIM], dt.float32)

    F = HID_DIM  # 64 (== EMB_DIM)

    with tile.TileContext(nc) as tc:
        with (
            tc.tile_pool(name="const", bufs=1) as cpool,
            tc.tile_pool(name="work", bufs=3) as wpool,
            tc.tile_pool(name="msg", bufs=16) as mpool,
            tc.tile_pool(name="ps", bufs=1, space="PSUM") as pspool,
            tc.tile_pool(name="psacc", bufs=1, space="PSUM") as paccpool,
            tc.tile_pool(name="ps1", bufs=2, space="PSUM") as pspool1,
        )    :
            idt = cpool.tile([P, P], dt.float32)
            nc.sync.dma_start(idt[:], ident[:])
            w1t = cpool.tile([IN_DIM, HID_DIM], dt.float32)
            nc.sync.dma_start(w1t[:], W1[:])
            w2t = cpool.tile([HID_DIM, EMB_DIM], dt.float32)
            nc.sync.dma_start(w2t[:], W2[:])
            dinvt = cpool.tile([P, n_chunks], dt.float32)
            nc.sync.dma_start(dinvt[:], dinvk[:])
            b1t = cpool.tile([P, HID_DIM], dt.float32)
            nc.sync.dma_start(b1t[:], b1r[:])
            b2t = cpool.tile([P, EMB_DIM], dt.float32)
            nc.sync.dma_start(b2t[:], b2r[:])
            St = cpool.tile([P, n_chunks, GMAX], dt.float32)
            nc.sync.dma_start(St[:], S_in.ap().rearrange("c p g -> p c g"))
            STt = cpool.tile([GMAX, n_chunks, P], dt.float32)
            nc.sync.dma_start(STt[:], ST_in.ap().rearrange("c g p -> g c p"))
            gnp1t = cpool.tile([GMAX, 3 * HID_DIM], dt.float32)
            nc.sync.dma_start(gnp1t[:], gnp1[:])
            gnp2t = cpool.tile([GMAX, 3 * EMB_DIM], dt.float32)
            nc.sync.dma_start(gnp2t[:], gnp2[:])
            cit = cpool.tile([GMAX, 1], dt.float32)
            nc.sync.dma_start(cit[:], cntinv[:])
            iot = cpool.tile([P, P], dt.float32)
            nc.sync.dma_start(iot[:], iota_in[:])
            gt = cpool.tile([P, n_blocks], dt.int32)
            nc.sync.dma_start(gt[:], gidx1[:])
            dvt = cpool.tile([P, n_blocks], dt.float32)
            nc.sync.dma_start(dvt[:], dstv1[:])

            # resident x2 (layer outputs, node-major chunks)
            x2t = cpool.tile([P, n_chunks, F], dt.float32)

            def dense_layer(src_chunks_ap, in_dim, wt, hp_dst):
                """hp_dst[c*128+p] = dinv * (x @ W) for each chunk."""
                for c in range(n_chunks):
                    src = src_chunks_ap(c)
                    if src.space == bass.MemorySpace.DRAM:
                        xc = wpool.tile([P, in_dim], dt.float32, tag="xc")
                        nc.sync.dma_start(xc[:], src)
                        src = xc[:]
                    xt_ps = pspool.tile([in_dim, P], dt.float32, tag="xtp")
                    nc.tensor.transpose(out=xt_ps[:], in_=src, identity=idt[:])
                    xts = wpool.tile([in_dim, P], dt.float32, tag="xts")
                    nc.vector.tensor_copy(xts[:], xt_ps[:])
                    h_ps = pspool1.tile([P, F], dt.float32, tag="hps")
                    nc.tensor.matmul(h_ps[:], lhsT=xts[:], rhs=wt[:],
                                     start=True, stop=True)
                    hc = wpool.tile([P, F], dt.float32, tag="hc")
                    nc.vector.tensor_tensor(
                        out=hc[:], in0=h_ps[:],
                        in1=dinvt[:, c:c + 1].to_broadcast([P, F]),
                        op=mybir.AluOpType.mult)
                    nc.sync.dma_start(hp_dst[c * P:(c + 1) * P, :], hc[:])

            c_fix = n_blocks // n_chunks

            def edge_phase(table_full, agg, hp_local):
                for w in range(n_chunks):
                    ps = pspool1.tile([P, F], dt.float32, tag="aggps")
                    hpw = wpool.tile([P, F], dt.float32, tag="hpw")
                    nc.sync.dma_start(hpw[:], hp_local[w * P:(w + 1) * P, :])
                    nc.tensor.matmul(ps[:], lhsT=idt[:], rhs=hpw[:],
                                     start=True, stop=False)
                    for j in range(c_fix):
                        b = w * c_fix + j
                        msg = mpool.tile([P, F], dt.float32, tag="msg")
                        nc.gpsimd.indirect_dma_start(
                            out=msg[:], out_offset=None,
                            in_=table_full[:],
                            in_offset=bass.IndirectOffsetOnAxis(
                                ap=gt[:, b:b + 1], axis=0))
                        S = mpool.tile([P, P], dt.float32, tag="sel")
                        nc.vector.tensor_tensor(
                            out=S[:],
                            in0=dvt[:, b:b + 1].to_broadcast([P, P]),
                            in1=iot[:], op=mybir.AluOpType.is_equal)
                        nc.tensor.matmul(ps[:], lhsT=S[:], rhs=msg[:],
                                         start=False, stop=(j == c_fix - 1))
                    outw = wpool.tile([P, F], dt.float32, tag="outw")
                    nc.vector.tensor_copy(outw[:], ps[:])
                    nc.sync.dma_start(agg[w * P:(w + 1) * P, :], outw[:])

            def post_layer(agg, bt, gnpt, fdim, out_tile):
                """out_tile[:, c, :] = relu(GN(dinv*agg + b)) ; also returns
                stats psum for later reuse. GN per graph via S matmuls."""
                # pass 1: y = dinv*agg + b (chunk-wise, keep resident), stats
                yt = out_tile
                st_ps = paccpool.tile([GMAX, 2 * fdim], dt.float32, tag="stats")
                for c in range(n_chunks):
                    t = wpool.tile([P, fdim], dt.float32, tag="pl1")
                    nc.sync.dma_start(t[:], agg[c * P:(c + 1) * P, :])
                    nc.vector.tensor_tensor(
                        out=t[:], in0=t[:],
                        in1=dinvt[:, c:c + 1].to_broadcast([P, fdim]),
                        op=mybir.AluOpType.mult)
                    nc.vector.tensor_tensor(
                        out=yt[:, c, :], in0=t[:], in1=bt[:],
                        op=mybir.AluOpType.add)
                    sq = wpool.tile([P, 2 * fdim], dt.float32, tag="sq")
                    nc.vector.tensor_copy(sq[:, 0:fdim], yt[:, c, :])
                    nc.vector.tensor_tensor(
                        out=sq[:, fdim:2 * fdim], in0=yt[:, c, :],
                        in1=yt[:, c, :], op=mybir.AluOpType.mult)
                    nc.tensor.matmul(st_ps[:], lhsT=St[:, c, :], rhs=sq[:],
                                     start=(c == 0), stop=(c == n_chunks - 1))
                # stats -> A, B   (alpha|weight|bias in gnpt)
                stats = wpool.tile([GMAX, 2 * fdim], dt.float32, tag="stf")
                nc.vector.tensor_tensor(
                    out=stats[:], in0=st_ps[:],
                    in1=cit[:, 0:1].to_broadcast([GMAX, 2 * fdim]),
                    op=mybir.AluOpType.mult)  # [mean | E[x^2]]
                mean = stats[:, 0:fdim]
                ex2 = stats[:, fdim:2 * fdim]
                alpha = gnpt[:, 0:fdim]
                weight = gnpt[:, fdim:2 * fdim]
                bias = gnpt[:, 2 * fdim:3 * fdim]
                am = wpool.tile([GMAX, fdim], dt.float32, tag="am")
                nc.vector.tensor_tensor(out=am[:], in0=alpha, in1=mean,
                                        op=mybir.AluOpType.mult)  # alpha*m
                var = wpool.tile([GMAX, fdim], dt.float32, tag="var")
                # var = E[x^2] - 2*am*m + am^2 = E[x^2] + am*(am - 2m)
                t2 = wpool.tile([GMAX, fdim], dt.float32, tag="t2")
                nc.vector.tensor_scalar(out=t2[:], in0=mean, scalar1=-2.0,
                                        scalar2=None,
                                        op0=mybir.AluOpType.mult)
                nc.vector.tensor_tensor(out=t2[:], in0=t2[:], in1=am[:],
                                        op=mybir.AluOpType.add)
                nc.vector.tensor_tensor(out=t2[:], in0=t2[:], in1=am[:],
                                        op=mybir.AluOpType.mult)
                nc.vector.tensor_tensor(out=var[:], in0=ex2, in1=t2[:],
                                        op=mybir.AluOpType.add)
                istd = wpool.tile([GMAX, fdim], dt.float32, tag="istd")
                nc.vector.tensor_scalar(out=istd[:], in0=var[:], scalar1=EPS,
                                        scalar2=None, op0=mybir.AluOpType.add)
                nc.scalar.activation(istd[:], istd[:],
                                     mybir.ActivationFunctionType.Sqrt)
                nc.vector.reciprocal(istd[:], istd[:])
                A = wpool.tile([GMAX, fdim], dt.float32, tag="A")
                nc.vector.tensor_tensor(out=A[:], in0=weight, in1=istd[:],
                                        op=mybir.AluOpType.mult)
                B = wpool.tile([GMAX, fdim], dt.float32, tag="B")
                nc.vector.tensor_tensor(out=B[:], in0=A[:], in1=am[:],
                                        op=mybir.AluOpType.mult)
                nc.vector.tensor_scalar(out=B[:], in0=B[:], scalar1=-1.0,
                                        scalar2=None, op0=mybir.AluOpType.mult)
                nc.vector.tensor_tensor(out=B[:], in0=B[:], in1=bias,
                                        op=mybir.AluOpType.add)
                AB = wpool.tile([GMAX, 2 * fdim], dt.float32, tag="AB")
                nc.vector.tensor_copy(AB[:, 0:fdim], A[:])
                nc.vector.tensor_copy(AB[:, fdim:2 * fdim], B[:])
                # pass 2: y = relu(y*Ae + Be) per chunk
                for c in range(n_chunks):
                    ab_ps = pspool.tile([P, 2 * fdim], dt.float32, tag="abps")
                    nc.tensor.matmul(ab_ps[:], lhsT=STt[:, c, :], rhs=AB[:],
                                     start=True, stop=True)
                    nc.vector.tensor_tensor(
                        out=yt[:, c, :], in0=yt[:, c, :],
                        in1=ab_ps[:, 0:fdim], op=mybir.AluOpType.mult)
                    nc.vector.tensor_tensor(
                        out=yt[:, c, :], in0=yt[:, c, :],
                        in1=ab_ps[:, fdim:2 * fdim], op=mybir.AluOpType.add)
                    nc.scalar.activation(yt[:, c, :], yt[:, c, :],
                                         mybir.ActivationFunctionType.Relu)

            # ---------------- layer 1 ----------------
            dense_layer(lambda c: xk[c * P:(c + 1) * P, :], IN_DIM, w1t, hp_loc)
            nc.gpsimd.collective_compute(
                "AllGather", mybir.AluOpType.bypass,
                replica_groups=[list(range(NCORES))],
                ins=[hp_loc.ap()], outs=[hp_full.ap()])
            edge_phase(hp_full, agg1, hp_loc)
            post_layer(agg1, b1t, gnp1t, HID_DIM, x2t)

            # ---------------- layer 2 ----------------
            dense_layer(lambda c: x2t[:, c, :], HID_DIM, w2t, hp2_loc)
            nc.gpsimd.collective_compute(
                "AllGather", mybir.AluOpType.bypass,
                replica_groups=[list(range(NCORES))],
                ins=[hp2_loc.ap()], outs=[hp2_full.ap()])
            edge_phase(hp2_full, agg2, hp2_loc)
            h2t = cpool.tile([P, n_chunks, F], dt.float32, tag="h2t")
            post_layer(agg2, b2t, gnp2t, EMB_DIM, h2t)

            # ---------------- pooling ----------------
            pl_ps = paccpool.tile([GMAX, EMB_DIM], dt.float32, tag="plps")
            for c in range(n_chunks):
                nc.tensor.matmul(pl_ps[:], lhsT=St[:, c, :], rhs=h2t[:, c, :],
                                 start=(c == 0), stop=(c == n_chunks - 1))
            plt = wpool.tile([GMAX, EMB_DIM], dt.float32, tag="plt")
            nc.vector.tensor_tensor(
                out=plt[:], in0=pl_ps[:],
                in1=cit[:, 0:1].to_broadcast([GMAX, EMB_DIM]),
                op=mybir.AluOpType.mult)
            nc.sync.dma_start(pool_out[:], plt[:])

    nc.compile()
    return nc


def kernel(x, edge_index, batch, W1, b1, alpha1, weight1, bias1,
           W2, b2, alpha2, weight2, bias2):
    x = np.asarray(x, np.float32)
    edge_index = np.asarray(edge_index, np.int32)
    batch = np.asarray(batch, np.int32)

    src, dst = edge_index[0].astype(np.int64), edge_index[1].astype(np.int64)
    deg = np.bincount(dst, minlength=NUM_NODES).astype(np.float32) + 1.0
    dinv = 1.0 / np.sqrt(deg)

    node_bounds, gstarts, graph_bounds = _shard_plan(batch)
    n_shard = int(np.max(node_bounds[1:] - node_bounds[:-1]))
    # +P guarantees every core has >=1 all-zero pad row (gather target for
    # padding slots in the edge schedule)
    n_shard = ((n_shard + P) // P) * P
    n_chunks = n_shard // P

    # remap node -> (core, local)
    core_of = np.searchsorted(node_bounds, np.arange(NUM_NODES), side="right") - 1
    local = np.arange(NUM_NODES) - node_bounds[core_of]
    remap = core_of * n_shard + local   # row in hp_full

    ecore = core_of[dst]
    per_core = []
    c_fix = 0
    for k in range(NCORES):
        m = ecore == k
        sl = src[m]
        dl = dst[m] - node_bounds[k]
        wcnt = np.bincount(dl // P, minlength=n_chunks)
        c_fix = max(c_fix, int(np.max((wcnt + P - 1) // P)))
        per_core.append((sl, dl))
    n_blocks = n_chunks * c_fix

    key = (n_shard, n_chunks, n_blocks)
    if key not in _CACHE:
        _CACHE[key] = _build_program(n_shard, n_chunks, n_blocks)
    nc = _CACHE[key]

    ident = np.eye(P, dtype=np.float32)

    in_maps = []
    pool_maps = []
    for k in range(NCORES):
        lo, hi = int(node_bounds[k]), int(node_bounds[k + 1])
        nk = hi - lo
        xk = np.zeros((n_shard, IN_DIM), np.float32)
        xk[:nk] = x[lo:hi]
        dv = np.zeros(n_shard, np.float32)
        dv[:nk] = dinv[lo:hi]
        dinvk = dv.reshape(n_chunks, P).T.copy()   # [P, n_chunks]

        sl, dl = per_core[k]
        srm = remap[sl]
        pad_src = k * n_shard + nk if nk < n_shard else k * n_shard
        gidx, dstv = _window_schedule(dl, srm, n_chunks, n_blocks // n_chunks,
                                      pad_src)
        g1 = gidx.reshape(n_blocks, P).T.copy()
        d1 = dstv.reshape(n_blocks, P).T.copy()

        glo, ghi = graph_bounds[k], graph_bounds[k + 1]
        ngr = ghi - glo
        assert ngr <= GMAX, ngr
        # S [n_chunks, P, GMAX] one-hot graph membership for local nodes
        gb = np.zeros(n_shard, np.int64)
        gb[:nk] = batch[lo:hi] - glo
        S = np.zeros((n_shard, GMAX), np.float32)
        S[np.arange(nk), gb[:nk]] = 1.0
        S3 = S.reshape(n_chunks, P, GMAX)
        ST3 = np.ascontiguousarray(S3.transpose(0, 2, 1))
        cnts = np.bincount(gb[:nk], minlength=GMAX).astype(np.float32)
        cntinv = (1.0 / np.maximum(cnts, 1.0)).reshape(GMAX, 1).astype(np.float32)

        gnp1 = np.concatenate([
            np.tile(alpha1, (GMAX, 1)), np.tile(weight1, (GMAX, 1)),
            np.tile(bias1, (GMAX, 1))], axis=1).astype(np.float32)
        gnp2 = np.concatenate([
            np.tile(alpha2, (GMAX, 1)), np.tile(weight2, (GMAX, 1)),
            np.tile(bias2, (GMAX, 1))], axis=1).astype(np.float32)

        in_maps.append({
            "xk": xk, "W1": np.asarray(W1, np.float32),
            "W2": np.asarray(W2, np.float32),
            "b1r": np.tile(np.asarray(b1, np.float32), (P, 1)),
            "b2r": np.tile(np.asarray(b2, np.float32), (P, 1)),
            "dinvk": np.ascontiguousarray(dinvk), "ident": ident,
            "gnp1": gnp1, "gnp2": gnp2, "cntinv": cntinv,
            "S_in": np.ascontiguousarray(S3), "ST_in": ST3,
            "gidx1": np.ascontiguousarray(g1),
            "dstv1": np.ascontiguousarray(d1),
            "iota_in": np.tile(np.arange(P, dtype=np.float32), (P, 1)),
        })
        pool_maps.append((glo, ghi))

    res = run_bass_kernel_spmd(nc, in_maps, list(range(NCORES)))

    out = np.zeros((NUM_GRAPHS, EMB_DIM), np.float32)
    for k in range(NCORES):
        glo, ghi = pool_maps[k]
        out[glo:ghi] = np.asarray(res.results[k]["pool_out"])[:ghi - glo]
    return out
